# revision 19
# baseline (speedup 1.0000x reference)
"""FFM layer kernel for Trainium2, data-parallel over batch on 8 NeuronCores.

The reference computes, for each sample b:
    x = [dense(13) | onehot(26 fields x 1000)]            # [B, 26013]
    linear = w0 + x @ w                                   # [B, 1]
    field_f = einsum('bf,fik->bik', x, v)                 # [B, 39, 8]
    inter = 0.5*((sum_i field_f)^2.sum(k) - (field_f^2).sum(i,k))
    out = linear + inter

Because x is one-hot in the sparse block, x @ [v|w] is a 26-row gather from
a [26013, 320] table (cols 0..311 = flattened v row, col 312 = w, 313.. pad)
plus a tiny dense [14]x[14,313] matmul (row 13 = ones row carrying w0 into
col 312).  Each core handles 512 samples as 4 tiles of 128; each tile's 26
rows/sample are fetched by one gpsimd dma_gather (3328 rows of 1280 B).
"""

import numpy as np

N_DENSE = 13
N_SPARSE = 26
ONEHOT = 1000
FIELD = 39
K = 8
FEAT = N_DENSE + N_SPARSE * ONEHOT  # 26013
B = 4096
NCORES = 8
BC = B // NCORES  # 512 samples per core
P = 128
NT = BC // P  # 4 tiles per core
D = FIELD * K  # 312
DW = D + 1  # 313 (col 312 carries the linear weight)
E = 384  # gathered fp16 row width, padded so the 768 B row is a multiple of 256
NI = N_SPARSE * P  # 3328 indices per tile gather
SQRT_HALF = 0.7071067811865476

_cached_nc = None


def _build_program():
    global _cached_nc
    if _cached_nc is not None:
        return _cached_nc

    import concourse.bacc as bacc
    import concourse.mybir as mybir
    from concourse.tile import TileContext
    from concourse import library_config

    nc = bacc.Bacc(
        "TRN2",
        debug=False,
        enable_asserts=False,
        target_bir_lowering=False,
        num_devices=NCORES,
        num_swdge_queues=4,
        dynamic_dma_scratch_size=32768,
    )
    f32 = mybir.dt.float32
    f16 = mybir.dt.float16
    i16 = mybir.dt.int16
    table = nc.dram_tensor("table", [FEAT, E], f16, kind="ExternalInput").ap()
    idx = nc.dram_tensor("idx", [P, NT * NI // 16], i16, kind="ExternalInput").ap()
    dnt = nc.dram_tensor("dnt", [N_DENSE + 1, BC], f32, kind="ExternalInput").ap()
    vdx = nc.dram_tensor("vdx", [N_DENSE + 1, DW], f32, kind="ExternalInput").ap()
    out = nc.dram_tensor("out", [BC, 1], f32, kind="ExternalOutput").ap()

    with TileContext(nc) as tc:
        with tc.tile_pool(name="const", bufs=1) as cpool, \
             tc.tile_pool(name="gath", bufs=NT) as gpool, \
             tc.tile_pool(name="work", bufs=2) as wpool, \
             tc.tile_pool(name="psum", bufs=2, space="PSUM") as ppool:
            nc.gpsimd.load_library(library_config.mlp)
            idx_sb = cpool.tile([P, NT * NI // 16], i16)
            nc.sync.dma_start(out=idx_sb[:], in_=idx[:])
            dnt_sb = cpool.tile([N_DENSE + 1, BC], f32)
            nc.sync.dma_start(out=dnt_sb[:], in_=dnt[:])
            vdx_sb = cpool.tile([N_DENSE + 1, DW], f32)
            nc.sync.dma_start(out=vdx_sb[:], in_=vdx[:])

            qn = 0
            for t in range(NT):
                # gh[h] covers 13 fields as two sub-gathers (7+6 fields) on
                # rotating SWDGE queues for finer stream interleaving
                HC = (NI // 2) // 16  # idx columns per 13-field half
                gh = []
                for h in range(2):
                    g = gpool.tile([P, 13 * E], f16, tag=f"g{h}")
                    g3 = g[:].rearrange("p (c e) -> p c e", e=E)
                    col = (2 * t + h) * HC
                    for (c0, nf) in ((0, 4), (4, 3), (7, 3), (10, 3)):
                        ni = nf * P
                        nc.gpsimd.dma_gather(
                            g3[:, c0:c0 + nf, :],
                            table[:],
                            idx_sb[:, col + c0 * 8:col + (c0 + nf) * 8],
                            ni,
                            ni,
                            E,
                            single_packet=False,
                            queue_num=qn % 4,
                        )
                        qn += 1
                    gh.append(g)
                # dense + w0 contribution: [128, 313]
                ps = ppool.tile([P, DW], f32, tag="ps")
                nc.tensor.matmul(
                    out=ps[:],
                    lhsT=dnt_sb[:, t * P:(t + 1) * P],
                    rhs=vdx_sb[:],
                    start=True,
                    stop=True,
                )
                # sum the 26 gathered rows with a contiguous pairwise tree
                # (fp32 tensor_tensor runs 1 elem/cycle; contiguous > strided)
                add = lambda o, a, b: nc.vector.tensor_tensor(
                    out=o, in0=a, in1=b, op=mybir.AluOpType.add
                )
                # fp16 halves collapse 13 blocks -> fp32 partials -> 1 block;
                # all adds use 313-wide views so the 71 pad columns per block
                # are never touched by the vector engine
                W = DW  # 313 payload columns per 384-wide block
                a6 = []
                for h in range(2):
                    g3 = gh[h][:].rearrange("p (c e) -> p c e", e=E)
                    a = wpool.tile([P, 6 * E], f32, tag=f"a6_{h}")
                    a3 = a[:].rearrange("p (c e) -> p c e", e=E)
                    add(a3[:, 0:6, 0:W], g3[:, 0:6, 0:W], g3[:, 6:12, 0:W])
                    add(a3[:, 0:3, 0:W], a3[:, 0:3, 0:W], a3[:, 3:6, 0:W])
                    a6.append(a)
                a03 = a6[0][:].rearrange("p (c e) -> p c e", e=E)
                a13 = a6[1][:].rearrange("p (c e) -> p c e", e=E)
                g03 = gh[0][:].rearrange("p (c e) -> p c e", e=E)
                g13 = gh[1][:].rearrange("p (c e) -> p c e", e=E)
                add(a03[:, 0:3, 0:W], a03[:, 0:3, 0:W], a13[:, 0:3, 0:W])
                add(a03[:, 0, 0:W], a03[:, 0, 0:W], a03[:, 1, 0:W])
                add(a03[:, 0, 0:W], a03[:, 0, 0:W], a03[:, 2, 0:W])
                add(a03[:, 0, 0:W], a03[:, 0, 0:W], g03[:, 12, 0:W])
                add(a03[:, 0, 0:W], a03[:, 0, 0:W], g13[:, 12, 0:W])
                tot = wpool.tile([P, DW], f32, tag="tot")
                add(tot[:], a6[0][:, :DW], ps[:])
                # s_k = sum_i field_f[i, k]: view [P, 8, 39], reduce innermost
                s8 = wpool.tile([P, K], f32, tag="s8")
                tv = tot[:, :D].rearrange("p (i k) -> p k i", k=K)
                nc.vector.reduce_sum(out=s8[:], in_=tv, axis=mybir.AxisListType.X)
                # 0.5 * sum of squares, fused on the scalar engine
                sq = wpool.tile([P, D], f32, tag="sq")
                h_sumsq = wpool.tile([P, 1], f32, tag="h_sumsq")
                nc.scalar.activation(
                    out=sq[:], in_=tot[:, :D],
                    func=mybir.ActivationFunctionType.Square,
                    scale=SQRT_HALF, accum_out=h_sumsq[:],
                )
                sq8 = wpool.tile([P, K], f32, tag="sq8")
                h_ssq = wpool.tile([P, 1], f32, tag="h_ssq")
                nc.scalar.activation(
                    out=sq8[:], in_=s8[:],
                    func=mybir.ActivationFunctionType.Square,
                    scale=SQRT_HALF, accum_out=h_ssq[:],
                )
                ot = wpool.tile([P, 1], f32, tag="ot")
                nc.vector.tensor_tensor(
                    out=ot[:], in0=h_ssq[:], in1=h_sumsq[:],
                    op=mybir.AluOpType.subtract,
                )
                add(ot[:], ot[:], tot[:, D:DW])
                nc.sync.dma_start(out=out[t * P:(t + 1) * P, :], in_=ot[:])

    nc.compile()
    _cached_nc = nc
    return nc


def _prepare_inputs(inputs, w0, w, v):
    dense = np.ascontiguousarray(inputs[:, :N_DENSE].astype(np.float32))
    idx = inputs[:, N_DENSE:].astype(np.int32)
    flat_idx = (N_DENSE + np.arange(N_SPARSE, dtype=np.int32) * ONEHOT)[None, :] + idx

    table = np.zeros((FEAT, E), np.float16)
    table[:, :D] = v.reshape(FEAT, D).astype(np.float16)
    table[:, D] = np.asarray(w, np.float32).reshape(FEAT).astype(np.float16)
    w0_row = np.zeros((1, DW), np.float32)
    w0_row[0, D] = np.asarray(w0, np.float32).reshape(-1)[0]
    vdx_top = np.concatenate(
        [v.reshape(FEAT, D)[:N_DENSE], np.asarray(w, np.float32).reshape(FEAT, 1)[:N_DENSE]],
        axis=1,
    ).astype(np.float32)
    vdx = np.ascontiguousarray(np.concatenate([vdx_top, w0_row], axis=0))

    in_maps = []
    for c in range(NCORES):
        sl = slice(c * BC, (c + 1) * BC)
        dnt = np.concatenate(
            [dense[sl].T, np.ones((1, BC), np.float32)], axis=0
        )  # [14, 512]
        # per tile t the gather consumes indices i = c*128 + p, laid out
        # int16 at [i % 16, i // 16] in the first 16 partitions, replicated
        # 8x down the partitions (one copy per Q7 core)
        fi = flat_idx[sl].astype(np.int16)  # [512, 26]
        blocks = []
        for t in range(NT):
            for h in range(2):
                # half h covers fields 13h..13h+12; order i = c_local*128 + p
                lin = fi[t * P:(t + 1) * P, 13 * h:13 * (h + 1)].T.reshape(NI // 2)
                blk = lin.reshape(NI // 32, 16).T  # [16, HNI/16]
                blocks.append(np.tile(blk, (8, 1)))  # [128, HNI/16]
        idx_buf = np.ascontiguousarray(np.concatenate(blocks, axis=1))
        in_maps.append(
            {
                "table": table,
                "idx": idx_buf,
                "dnt": np.ascontiguousarray(dnt),
                "vdx": vdx,
            }
        )
    return in_maps


def kernel(**inputs):
    from concourse import bass_utils

    nc = _build_program()
    in_maps = _prepare_inputs(
        np.asarray(inputs["inputs"]),
        np.asarray(inputs["w0"]),
        np.asarray(inputs["w"]),
        np.asarray(inputs["v"]),
    )
    res = bass_utils.run_bass_kernel_spmd(nc, in_maps, core_ids=list(range(NCORES)))
    outs = [np.asarray(res.results[c]["out"]) for c in range(NCORES)]
    return np.concatenate(outs, axis=0).astype(np.float32)


# revision 20
# speedup vs baseline: 1.0090x; 1.0090x over previous
"""FFM layer kernel for Trainium2, data-parallel over batch on 8 NeuronCores.

The reference computes, for each sample b:
    x = [dense(13) | onehot(26 fields x 1000)]            # [B, 26013]
    linear = w0 + x @ w                                   # [B, 1]
    field_f = einsum('bf,fik->bik', x, v)                 # [B, 39, 8]
    inter = 0.5*((sum_i field_f)^2.sum(k) - (field_f^2).sum(i,k))
    out = linear + inter

Because x is one-hot in the sparse block, x @ [v|w] is a 26-row gather from
a [26013, 320] table (cols 0..311 = flattened v row, col 312 = w, 313.. pad)
plus a tiny dense [14]x[14,313] matmul (row 13 = ones row carrying w0 into
col 312).  Each core handles 512 samples as 4 tiles of 128; each tile's 26
rows/sample are fetched by one gpsimd dma_gather (3328 rows of 1280 B).
"""

import numpy as np

N_DENSE = 13
N_SPARSE = 26
ONEHOT = 1000
FIELD = 39
K = 8
FEAT = N_DENSE + N_SPARSE * ONEHOT  # 26013
B = 4096
NCORES = 8
BC = B // NCORES  # 512 samples per core
P = 128
NT = BC // P  # 4 tiles per core
D = FIELD * K  # 312
DW = D + 1  # 313 (col 312 carries the linear weight)
E = 384  # gathered fp16 row width, padded so the 768 B row is a multiple of 256
NI = N_SPARSE * P  # 3328 indices per tile gather
SQRT_HALF = 0.7071067811865476

_cached_nc = None


def _build_program():
    global _cached_nc
    if _cached_nc is not None:
        return _cached_nc

    import concourse.bacc as bacc
    import concourse.mybir as mybir
    from concourse.tile import TileContext
    from concourse import library_config

    nc = bacc.Bacc(
        "TRN2",
        debug=False,
        enable_asserts=False,
        target_bir_lowering=False,
        num_devices=NCORES,
        num_swdge_queues=4,
        dynamic_dma_scratch_size=32768,
    )
    f32 = mybir.dt.float32
    f16 = mybir.dt.float16
    i16 = mybir.dt.int16
    table = nc.dram_tensor("table", [FEAT, E], f16, kind="ExternalInput").ap()
    idx = nc.dram_tensor("idx", [P, NT * NI // 16], i16, kind="ExternalInput").ap()
    dnt = nc.dram_tensor("dnt", [N_DENSE + 1, BC], f32, kind="ExternalInput").ap()
    vdx = nc.dram_tensor("vdx", [N_DENSE + 1, DW], f32, kind="ExternalInput").ap()
    out = nc.dram_tensor("out", [BC, 1], f32, kind="ExternalOutput").ap()

    with TileContext(nc) as tc:
        with tc.tile_pool(name="const", bufs=1) as cpool, \
             tc.tile_pool(name="gath", bufs=NT) as gpool, \
             tc.tile_pool(name="work", bufs=2) as wpool, \
             tc.tile_pool(name="psum", bufs=2, space="PSUM") as ppool:
            nc.gpsimd.load_library(library_config.mlp)
            idx_sb = cpool.tile([P, NT * NI // 16], i16)
            nc.sync.dma_start(out=idx_sb[:], in_=idx[:])
            dnt_sb = cpool.tile([N_DENSE + 1, BC], f32)
            nc.sync.dma_start(out=dnt_sb[:], in_=dnt[:])
            vdx_sb = cpool.tile([N_DENSE + 1, DW], f32)
            nc.sync.dma_start(out=vdx_sb[:], in_=vdx[:])

            qn = 0
            for t in range(NT):
                # gh[h] covers 13 fields as two sub-gathers (7+6 fields) on
                # rotating SWDGE queues for finer stream interleaving
                HC = (NI // 2) // 16  # idx columns per 13-field half
                gh = []
                for h in range(2):
                    g = gpool.tile([P, 13 * E], f16, tag=f"g{h}")
                    g3 = g[:].rearrange("p (c e) -> p c e", e=E)
                    col = (2 * t + h) * HC
                    for (c0, nf) in ((0, 5), (5, 4), (9, 4)):
                        ni = nf * P
                        nc.gpsimd.dma_gather(
                            g3[:, c0:c0 + nf, :],
                            table[:],
                            idx_sb[:, col + c0 * 8:col + (c0 + nf) * 8],
                            ni,
                            ni,
                            E,
                            single_packet=False,
                            queue_num=qn % 4,
                        )
                        qn += 1
                    gh.append(g)
                # dense + w0 contribution: [128, 313]
                ps = ppool.tile([P, DW], f32, tag="ps")
                nc.tensor.matmul(
                    out=ps[:],
                    lhsT=dnt_sb[:, t * P:(t + 1) * P],
                    rhs=vdx_sb[:],
                    start=True,
                    stop=True,
                )
                # sum the 26 gathered rows with a contiguous pairwise tree
                # (fp32 tensor_tensor runs 1 elem/cycle; contiguous > strided)
                add = lambda o, a, b: nc.vector.tensor_tensor(
                    out=o, in0=a, in1=b, op=mybir.AluOpType.add
                )
                # fp16 halves collapse 13 blocks -> fp32 partials -> 1 block;
                # all adds use 313-wide views so the 71 pad columns per block
                # are never touched by the vector engine
                W = DW  # 313 payload columns per 384-wide block
                a6 = []
                for h in range(2):
                    g3 = gh[h][:].rearrange("p (c e) -> p c e", e=E)
                    a = wpool.tile([P, 6 * E], f32, tag=f"a6_{h}")
                    a3 = a[:].rearrange("p (c e) -> p c e", e=E)
                    add(a3[:, 0:6, 0:W], g3[:, 0:6, 0:W], g3[:, 6:12, 0:W])
                    add(a3[:, 0:3, 0:W], a3[:, 0:3, 0:W], a3[:, 3:6, 0:W])
                    a6.append(a)
                a03 = a6[0][:].rearrange("p (c e) -> p c e", e=E)
                a13 = a6[1][:].rearrange("p (c e) -> p c e", e=E)
                g03 = gh[0][:].rearrange("p (c e) -> p c e", e=E)
                g13 = gh[1][:].rearrange("p (c e) -> p c e", e=E)
                add(a03[:, 0:3, 0:W], a03[:, 0:3, 0:W], a13[:, 0:3, 0:W])
                add(a03[:, 0, 0:W], a03[:, 0, 0:W], a03[:, 1, 0:W])
                add(a03[:, 0, 0:W], a03[:, 0, 0:W], a03[:, 2, 0:W])
                add(a03[:, 0, 0:W], a03[:, 0, 0:W], g03[:, 12, 0:W])
                add(a03[:, 0, 0:W], a03[:, 0, 0:W], g13[:, 12, 0:W])
                tot = wpool.tile([P, DW], f32, tag="tot")
                add(tot[:], a6[0][:, :DW], ps[:])
                # s_k = sum_i field_f[i, k]: view [P, 8, 39], reduce innermost
                s8 = wpool.tile([P, K], f32, tag="s8")
                tv = tot[:, :D].rearrange("p (i k) -> p k i", k=K)
                nc.vector.reduce_sum(out=s8[:], in_=tv, axis=mybir.AxisListType.X)
                # 0.5 * sum of squares, fused on the scalar engine
                sq = wpool.tile([P, D], f32, tag="sq")
                h_sumsq = wpool.tile([P, 1], f32, tag="h_sumsq")
                nc.scalar.activation(
                    out=sq[:], in_=tot[:, :D],
                    func=mybir.ActivationFunctionType.Square,
                    scale=SQRT_HALF, accum_out=h_sumsq[:],
                )
                sq8 = wpool.tile([P, K], f32, tag="sq8")
                h_ssq = wpool.tile([P, 1], f32, tag="h_ssq")
                nc.scalar.activation(
                    out=sq8[:], in_=s8[:],
                    func=mybir.ActivationFunctionType.Square,
                    scale=SQRT_HALF, accum_out=h_ssq[:],
                )
                ot = wpool.tile([P, 1], f32, tag="ot")
                nc.vector.tensor_tensor(
                    out=ot[:], in0=h_ssq[:], in1=h_sumsq[:],
                    op=mybir.AluOpType.subtract,
                )
                add(ot[:], ot[:], tot[:, D:DW])
                nc.sync.dma_start(out=out[t * P:(t + 1) * P, :], in_=ot[:])

    nc.compile()
    _cached_nc = nc
    return nc


def _prepare_inputs(inputs, w0, w, v):
    dense = np.ascontiguousarray(inputs[:, :N_DENSE].astype(np.float32))
    idx = inputs[:, N_DENSE:].astype(np.int32)
    flat_idx = (N_DENSE + np.arange(N_SPARSE, dtype=np.int32) * ONEHOT)[None, :] + idx

    table = np.zeros((FEAT, E), np.float16)
    table[:, :D] = v.reshape(FEAT, D).astype(np.float16)
    table[:, D] = np.asarray(w, np.float32).reshape(FEAT).astype(np.float16)
    w0_row = np.zeros((1, DW), np.float32)
    w0_row[0, D] = np.asarray(w0, np.float32).reshape(-1)[0]
    vdx_top = np.concatenate(
        [v.reshape(FEAT, D)[:N_DENSE], np.asarray(w, np.float32).reshape(FEAT, 1)[:N_DENSE]],
        axis=1,
    ).astype(np.float32)
    vdx = np.ascontiguousarray(np.concatenate([vdx_top, w0_row], axis=0))

    in_maps = []
    for c in range(NCORES):
        sl = slice(c * BC, (c + 1) * BC)
        dnt = np.concatenate(
            [dense[sl].T, np.ones((1, BC), np.float32)], axis=0
        )  # [14, 512]
        # per tile t the gather consumes indices i = c*128 + p, laid out
        # int16 at [i % 16, i // 16] in the first 16 partitions, replicated
        # 8x down the partitions (one copy per Q7 core)
        fi = flat_idx[sl].astype(np.int16)  # [512, 26]
        blocks = []
        for t in range(NT):
            for h in range(2):
                # half h covers fields 13h..13h+12; order i = c_local*128 + p
                lin = fi[t * P:(t + 1) * P, 13 * h:13 * (h + 1)].T.reshape(NI // 2)
                blk = lin.reshape(NI // 32, 16).T  # [16, HNI/16]
                blocks.append(np.tile(blk, (8, 1)))  # [128, HNI/16]
        idx_buf = np.ascontiguousarray(np.concatenate(blocks, axis=1))
        in_maps.append(
            {
                "table": table,
                "idx": idx_buf,
                "dnt": np.ascontiguousarray(dnt),
                "vdx": vdx,
            }
        )
    return in_maps


def kernel(**inputs):
    from concourse import bass_utils

    nc = _build_program()
    in_maps = _prepare_inputs(
        np.asarray(inputs["inputs"]),
        np.asarray(inputs["w0"]),
        np.asarray(inputs["w"]),
        np.asarray(inputs["v"]),
    )
    res = bass_utils.run_bass_kernel_spmd(nc, in_maps, core_ids=list(range(NCORES)))
    outs = [np.asarray(res.results[c]["out"]) for c in range(NCORES)]
    return np.concatenate(outs, axis=0).astype(np.float32)


# revision 25
# speedup vs baseline: 1.0206x; 1.0115x over previous
"""FFM layer kernel for Trainium2, data-parallel over batch on 8 NeuronCores.

The reference computes, for each sample b:
    x = [dense(13) | onehot(26 fields x 1000)]            # [B, 26013]
    linear = w0 + x @ w                                   # [B, 1]
    field_f = einsum('bf,fik->bik', x, v)                 # [B, 39, 8]
    inter = 0.5*((sum_i field_f)^2.sum(k) - (field_f^2).sum(i,k))
    out = linear + inter

Because x is one-hot in the sparse block, x @ [v|w] is a 26-row gather from
an fp16 [26013, 384] table (cols 0..311 = flattened v row, col 312 = w,
313.. pad so each row is 768 B, a multiple of 256) plus a tiny fp32 dense
[14]x[14,313] matmul (row 13 = ones row carrying w0 into col 312).  Each
core handles 512 samples as 4 tiles of 128; each tile's 26 rows/sample are
fetched by six dma_gather calls (5/4/4 fields per 13-field half) spread
over 4 SWDGE queues.  Raw bacc with manual semaphores — no TileContext —
to avoid the ~35 us Tile prelude/epilogue.
"""

import numpy as np

N_DENSE = 13
N_SPARSE = 26
ONEHOT = 1000
FIELD = 39
K = 8
FEAT = N_DENSE + N_SPARSE * ONEHOT  # 26013
B = 4096
NCORES = 8
BC = B // NCORES  # 512 samples per core
P = 128
NT = BC // P  # 4 tiles per core
D = FIELD * K  # 312
DW = D + 1  # 313 (col 312 carries the linear weight)
E = 384  # gathered fp16 row width (768 B, multiple of 256)
NI = N_SPARSE * P  # 3328 gathered rows per tile
HC = (NI // 2) // 16  # idx columns per 13-field half
SPLITS = ((0, 5), (5, 4), (9, 4))  # sub-gathers per half
SQRT_HALF = 0.7071067811865476

_cached_nc = None


def _build_program():
    global _cached_nc
    if _cached_nc is not None:
        return _cached_nc

    import concourse.bacc as bacc
    import concourse.mybir as mybir
    from concourse import library_config

    nc = bacc.Bacc(
        "TRN2",
        debug=False,
        enable_asserts=False,
        target_bir_lowering=False,
        num_devices=NCORES,
        num_swdge_queues=4,
        dynamic_dma_scratch_size=32768,
    )
    f32 = mybir.dt.float32
    f16 = mybir.dt.float16
    i16 = mybir.dt.int16
    add_op = mybir.AluOpType.add

    table = nc.dram_tensor("table", [FEAT, E], f16, kind="ExternalInput").ap()
    idx = nc.dram_tensor("idx", [P, NT * NI // 16], i16, kind="ExternalInput").ap()
    dnt = nc.dram_tensor("dnt", [N_DENSE + 1, BC], f32, kind="ExternalInput").ap()
    vdx = nc.dram_tensor("vdx", [N_DENSE + 1, DW], f32, kind="ExternalInput").ap()
    out = nc.dram_tensor("out", [BC, 1], f32, kind="ExternalOutput").ap()

    idx_sb = nc.alloc_sbuf_tensor("idx_sb", [P, NT * NI // 16], i16).ap()
    dnt_sb = nc.alloc_sbuf_tensor("dnt_sb", [N_DENSE + 1, BC], f32).ap()
    vdx_sb = nc.alloc_sbuf_tensor("vdx_sb", [N_DENSE + 1, DW], f32).ap()
    g_sb = [
        [nc.alloc_sbuf_tensor(f"g{t}_{h}", [P, 13 * E], f16).ap() for h in range(2)]
        for t in range(NT)
    ]
    # two alternating fp32 partial-sum sets (vector-engine-private, so the
    # engine's own program order serializes reuse)
    a_sb = [
        [nc.alloc_sbuf_tensor(f"a{s}_{h}", [P, 6 * E], f32).ap() for h in range(2)]
        for s in range(2)
    ]
    tot_sb = [nc.alloc_sbuf_tensor(f"tot{t}", [P, DW], f32).ap() for t in range(NT)]
    s8_sb = [nc.alloc_sbuf_tensor(f"s8_{t}", [P, K], f32).ap() for t in range(NT)]
    sq_sb = [nc.alloc_sbuf_tensor(f"sq{s}", [P, D], f32).ap() for s in range(2)]
    sq8_sb = [nc.alloc_sbuf_tensor(f"sq8_{s}", [P, K], f32).ap() for s in range(2)]
    h1_sb = [nc.alloc_sbuf_tensor(f"h1_{t}", [P, 1], f32).ap() for t in range(NT)]
    h2_sb = [nc.alloc_sbuf_tensor(f"h2_{t}", [P, 1], f32).ap() for t in range(NT)]
    ot_sb = [nc.alloc_sbuf_tensor(f"ot{t}", [P, 1], f32).ap() for t in range(NT)]
    ps_ps = [nc.alloc_psum_tensor(f"ps{t}", [P, DW], f32).ap() for t in range(NT)]

    io_idx = nc.alloc_semaphore("io_idx")  # idx load x 16
    io_dv = nc.alloc_semaphore("io_dv")    # dnt+vdx loads x 16 each
    st = nc.alloc_semaphore("st")      # output stores x 16
    # one sem per sub-gather: a DMA sem may only be updated from one SWDGE
    # queue, so the 3 sub-gathers of a tile-half can't share one
    gs = [
        [[nc.alloc_semaphore(f"gs{t}_{h}_{k}") for k in range(len(SPLITS))]
         for h in range(2)]
        for t in range(NT)
    ]
    mm = nc.alloc_semaphore("mm")      # matmul done (per tile)
    ve = nc.alloc_semaphore("ve")      # tot+s8 ready for scalar engine
    ac = nc.alloc_semaphore("ac")      # scalar squares done (2 per tile)
    dn = nc.alloc_semaphore("dn")      # ot ready for store
    vv = nc.alloc_semaphore("vv")      # vector-engine same-engine RAW ordering

    with nc.Block() as block:

        @block.sync
        def _(sync):
            sync.dma_start(idx_sb[:], idx[:]).then_inc(io_idx, 16)
            sync.dma_start(dnt_sb[:], dnt[:]).then_inc(io_dv, 16)
            sync.dma_start(vdx_sb[:], vdx[:]).then_inc(io_dv, 16)
            for t in range(NT):
                sync.wait_ge(dn, t + 1)
                sync.dma_start(out[t * P:(t + 1) * P, :], ot_sb[t][:]).then_inc(st, 16)
            sync.wait_ge(st, 16 * NT)

        @block.gpsimd
        def _(gpsimd):
            from concourse import library_config as lc

            gpsimd.load_library(lc.mlp)
            gpsimd.wait_ge(io_idx, 16)
            qn = 0
            for t in range(NT):
                for h in range(2):
                    g3 = g_sb[t][h].rearrange("p (c e) -> p c e", e=E)
                    col = (2 * t + h) * HC
                    for k, (c0, nf) in enumerate(SPLITS):
                        gpsimd.dma_gather(
                            g3[:, c0:c0 + nf, :],
                            table[:],
                            idx_sb[:, col + c0 * 8:col + (c0 + nf) * 8],
                            nf * P,
                            nf * P,
                            E,
                            single_packet=False,
                            queue_num=qn % 4,
                        ).then_inc(gs[t][h][k], 16)
                        qn += 1

        @block.tensor
        def _(tensor):
            tensor.wait_ge(io_dv, 32)
            for t in range(NT):
                nc.tensor.matmul(
                    out=ps_ps[t][:],
                    lhsT=dnt_sb[:, t * P:(t + 1) * P],
                    rhs=vdx_sb[:],
                    start=True,
                    stop=True,
                ).then_inc(mm, 1)

        @block.vector
        def _(vector):
            W = DW  # 313-wide views skip the pad columns
            # vv counts completed vector ops (13 per tile) so dependent
            # same-engine ops can wait out the deep DVE pipeline
            for t in range(NT):
                base = 11 * t
                s = t % 2
                a3h = []
                g3h = []
                for h in range(2):
                    for k in range(len(SPLITS)):
                        vector.wait_ge(gs[t][h][k], 16)
                    g3 = g_sb[t][h].rearrange("p (c e) -> p c e", e=E)
                    a3 = a_sb[s][h].rearrange("p (c e) -> p c e", e=E)
                    # op 1/2: L1 adds for the two halves (independent)
                    nc.vector.tensor_tensor(
                        out=a3[:, 0:6, 0:W], in0=g3[:, 0:6, 0:W],
                        in1=g3[:, 6:12, 0:W], op=add_op,
                    ).then_inc(vv, 1)
                    a3h.append(a3)
                    g3h.append(g3)
                for h in range(2):
                    # op 3/4: L2 in-place, needs op 1/2 respectively
                    vector.wait_ge(vv, base + 1 + h)
                    nc.vector.tensor_tensor(
                        out=a3h[h][:, 0:3, 0:W], in0=a3h[h][:, 0:3, 0:W],
                        in1=a3h[h][:, 3:6, 0:W], op=add_op,
                    ).then_inc(vv, 1)
                a03, a13 = a3h
                g03, g13 = g3h
                chain = [
                    (a03[:, 0:3, 0:W], a13[:, 0:3, 0:W]),  # op 5
                    (a03[:, 0, 0:W], a03[:, 1, 0:W]),      # op 6
                    (a03[:, 0, 0:W], a03[:, 2, 0:W]),      # op 7
                    (a03[:, 0, 0:W], g03[:, 12, 0:W]),     # op 8
                    (a03[:, 0, 0:W], g13[:, 12, 0:W]),     # op 9
                ]
                for j, (dst, src_) in enumerate(chain):
                    vector.wait_ge(vv, base + 4 + j)
                    nc.vector.tensor_tensor(
                        out=dst, in0=dst, in1=src_, op=add_op,
                    ).then_inc(vv, 1)
                vector.wait_ge(mm, t + 1)
                vector.wait_ge(vv, base + 9)
                nc.vector.tensor_tensor(       # op 10
                    out=tot_sb[t][:], in0=a_sb[s][0][:, :DW],
                    in1=ps_ps[t][:], op=add_op,
                ).then_inc(vv, 1)
                tv = tot_sb[t][:, :D].rearrange("p (i k) -> p k i", k=K)
                vector.wait_ge(vv, base + 10)
                nc.vector.reduce_sum(          # op 11 (inc carried by ve)
                    out=s8_sb[t][:], in_=tv, axis=mybir.AxisListType.X
                ).then_inc(ve, 1)
                vector.wait_ge(ac, 2 * (t + 1))
                nc.vector.tensor_tensor(       # op 12
                    out=ot_sb[t][:], in0=h2_sb[t][:], in1=h1_sb[t][:],
                    op=mybir.AluOpType.subtract,
                ).then_inc(vv, 1)
                vector.wait_ge(vv, base + 11)
                nc.vector.tensor_tensor(       # op 13 (inc carried by dn)
                    out=ot_sb[t][:], in0=ot_sb[t][:], in1=tot_sb[t][:, D:DW],
                    op=add_op,
                ).then_inc(dn, 1)

        @block.scalar
        def _(scalar):
            for t in range(NT):
                s = t % 2
                scalar.wait_ge(ve, t + 1)
                nc.scalar.activation(
                    out=sq_sb[s][:], in_=tot_sb[t][:, :D],
                    func=mybir.ActivationFunctionType.Square,
                    scale=SQRT_HALF, accum_out=h1_sb[t][:],
                ).then_inc(ac, 1)
                nc.scalar.activation(
                    out=sq8_sb[s][:], in_=s8_sb[t][:],
                    func=mybir.ActivationFunctionType.Square,
                    scale=SQRT_HALF, accum_out=h2_sb[t][:],
                ).then_inc(ac, 1)

    nc.compile()
    _cached_nc = nc
    return nc


def _prepare_inputs(inputs, w0, w, v):
    dense = np.ascontiguousarray(inputs[:, :N_DENSE].astype(np.float32))
    idx = inputs[:, N_DENSE:].astype(np.int32)
    flat_idx = (N_DENSE + np.arange(N_SPARSE, dtype=np.int32) * ONEHOT)[None, :] + idx

    table = np.zeros((FEAT, E), np.float16)
    table[:, :D] = v.reshape(FEAT, D).astype(np.float16)
    table[:, D] = np.asarray(w, np.float32).reshape(FEAT).astype(np.float16)
    w0_row = np.zeros((1, DW), np.float32)
    w0_row[0, D] = np.asarray(w0, np.float32).reshape(-1)[0]
    vdx_top = np.concatenate(
        [v.reshape(FEAT, D)[:N_DENSE], np.asarray(w, np.float32).reshape(FEAT, 1)[:N_DENSE]],
        axis=1,
    ).astype(np.float32)
    vdx = np.ascontiguousarray(np.concatenate([vdx_top, w0_row], axis=0))

    in_maps = []
    for c in range(NCORES):
        sl = slice(c * BC, (c + 1) * BC)
        dnt = np.concatenate(
            [dense[sl].T, np.ones((1, BC), np.float32)], axis=0
        )  # [14, 512]
        # per tile t and half h the gathers consume indices i = c_local*128+p,
        # laid out int16 at [i % 16, i // 16] in the first 16 partitions,
        # replicated 8x down the partitions (one copy per Q7 core)
        fi = flat_idx[sl].astype(np.int16)  # [512, 26]
        blocks = []
        for t in range(NT):
            for h in range(2):
                lin = fi[t * P:(t + 1) * P, 13 * h:13 * (h + 1)].T.reshape(NI // 2)
                blk = lin.reshape(NI // 32, 16).T  # [16, HC]
                blocks.append(np.tile(blk, (8, 1)))  # [128, HC]
        idx_buf = np.ascontiguousarray(np.concatenate(blocks, axis=1))
        in_maps.append(
            {
                "table": table,
                "idx": idx_buf,
                "dnt": np.ascontiguousarray(dnt),
                "vdx": vdx,
            }
        )
    return in_maps


def kernel(**inputs):
    from concourse import bass_utils

    nc = _build_program()
    in_maps = _prepare_inputs(
        np.asarray(inputs["inputs"]),
        np.asarray(inputs["w0"]),
        np.asarray(inputs["w"]),
        np.asarray(inputs["v"]),
    )
    res = bass_utils.run_bass_kernel_spmd(nc, in_maps, core_ids=list(range(NCORES)))
    outs = [np.asarray(res.results[c]["out"]) for c in range(NCORES)]
    return np.concatenate(outs, axis=0).astype(np.float32)


# revision 31
# speedup vs baseline: 1.0729x; 1.0512x over previous
"""FFM layer kernel for Trainium2, data-parallel over batch on 8 NeuronCores.

The reference computes, for each sample b:
    x = [dense(13) | onehot(26 fields x 1000)]            # [B, 26013]
    linear = w0 + x @ w                                   # [B, 1]
    field_f = einsum('bf,fik->bik', x, v)                 # [B, 39, 8]
    inter = 0.5*((sum_i field_f)^2.sum(k) - (field_f^2).sum(i,k))
    out = linear + inter

Because x is one-hot in the sparse block, x @ [v|w] is a 26-row gather from
an fp16 [26013, 384] table (cols 0..311 = flattened v row, col 312 = w,
313.. pad so each row is 768 B, a multiple of 256) plus a tiny fp32 dense
[14]x[14,313] matmul (row 13 = ones row carrying w0 into col 312).  Each
core handles 512 samples as 4 tiles of 128; each tile's 26 rows/sample are
fetched by six dma_gather calls (5/4/4 fields per 13-field half) spread
over 4 SWDGE queues.  Raw bacc with manual semaphores — no TileContext —
to avoid the ~35 us Tile prelude/epilogue.
"""

import numpy as np

N_DENSE = 13
N_SPARSE = 26
ONEHOT = 1000
FIELD = 39
K = 8
FEAT = N_DENSE + N_SPARSE * ONEHOT  # 26013
B = 4096
NCORES = 8
BC = B // NCORES  # 512 samples per core
P = 128
NT = BC // P  # 4 tiles per core
D = FIELD * K  # 312
DW = D + 1  # 313 (col 312 carries the linear weight)
E = 384  # gathered fp16 row width (768 B, multiple of 256)
NI = N_SPARSE * P  # 3328 gathered rows per tile
HC = (NI // 2) // 16  # idx columns per 13-field half
SPLITS = ((0, 5), (5, 4), (9, 4))  # sub-gathers per half
SQRT_HALF = 0.7071067811865476

_cached_nc = None


def _build_program():
    global _cached_nc
    if _cached_nc is not None:
        return _cached_nc

    import concourse.bacc as bacc
    import concourse.mybir as mybir
    from concourse import library_config

    nc = bacc.Bacc(
        "TRN2",
        debug=False,
        enable_asserts=False,
        target_bir_lowering=False,
        num_devices=NCORES,
        num_swdge_queues=4,
        dynamic_dma_scratch_size=32768,
    )
    f32 = mybir.dt.float32
    f16 = mybir.dt.float16
    i16 = mybir.dt.int16
    add_op = mybir.AluOpType.add

    table = nc.dram_tensor("table", [FEAT, E], f16, kind="ExternalInput").ap()
    idx = nc.dram_tensor("idx", [P, NT * NI // 16], i16, kind="ExternalInput").ap()
    dnt = nc.dram_tensor("dnt", [N_DENSE + 1, BC], f32, kind="ExternalInput").ap()
    vdx = nc.dram_tensor("vdx", [N_DENSE + 1, DW], f32, kind="ExternalInput").ap()
    ident = nc.dram_tensor("ident", [P, P], f32, kind="ExternalInput").ap()
    out = nc.dram_tensor("out", [BC, 1], f32, kind="ExternalOutput").ap()
    outT = out.rearrange("(t p) o -> t (p o)", t=NT)

    idx_sb = nc.alloc_sbuf_tensor("idx_sb", [P, NT * NI // 16], i16).ap()
    dnt_sb = nc.alloc_sbuf_tensor("dnt_sb", [N_DENSE + 1, BC], f32).ap()
    vdx_sb = nc.alloc_sbuf_tensor("vdx_sb", [N_DENSE + 1, DW], f32).ap()
    id_sb = nc.alloc_sbuf_tensor("id_sb", [P, P], f32).ap()
    g_sb = [
        [nc.alloc_sbuf_tensor(f"g{t}_{h}", [P, 13 * E], f16).ap() for h in range(2)]
        for t in range(NT)
    ]
    # two alternating fp32 partial-sum sets (vector-engine-private, so the
    # engine's own program order serializes reuse)
    a_sb = [
        [nc.alloc_sbuf_tensor(f"a{s}_{h}", [P, 6 * E], f32).ap() for h in range(2)]
        for s in range(2)
    ]
    tot_sb = [nc.alloc_sbuf_tensor(f"tot{t}", [P, DW], f32).ap() for t in range(NT)]
    s8_sb = [nc.alloc_sbuf_tensor(f"s8_{t}", [P, K], f32).ap() for t in range(NT)]
    sq_sb = [nc.alloc_sbuf_tensor(f"sq{s}", [P, D], f32).ap() for s in range(2)]
    sq8_sb = [nc.alloc_sbuf_tensor(f"sq8_{s}", [P, K], f32).ap() for s in range(2)]
    h1_sb = [nc.alloc_sbuf_tensor(f"h1_{t}", [P, 1], f32).ap() for t in range(NT)]
    h2_sb = [nc.alloc_sbuf_tensor(f"h2_{t}", [P, 1], f32).ap() for t in range(NT)]
    ot4_sb = nc.alloc_sbuf_tensor("ot4", [P, NT], f32).ap()
    ot_sb = [ot4_sb[:, t:t + 1] for t in range(NT)]
    otT_sb = nc.alloc_sbuf_tensor("otT", [NT, P], f32).ap()
    ps_ps = [nc.alloc_psum_tensor(f"ps{t}", [P, DW], f32).ap() for t in range(NT)]
    pst_ps = nc.alloc_psum_tensor("psT", [NT, P], f32).ap()

    io_idx = nc.alloc_semaphore("io_idx")  # idx load x 16
    io_dv = nc.alloc_semaphore("io_dv")    # dnt+vdx loads x 16 each
    st = nc.alloc_semaphore("st")      # output stores x 16
    # one sem per sub-gather: a DMA sem may only be updated from one SWDGE
    # queue, so the 3 sub-gathers of a tile-half can't share one
    gs = [
        [[nc.alloc_semaphore(f"gs{t}_{h}_{k}") for k in range(len(SPLITS))]
         for h in range(2)]
        for t in range(NT)
    ]
    mm = nc.alloc_semaphore("mm")      # matmul done (per tile)
    ve = nc.alloc_semaphore("ve")      # tot+s8 ready for scalar engine
    ac = nc.alloc_semaphore("ac")      # scalar squares done (2 per tile)
    dn = nc.alloc_semaphore("dn")      # ot ready for store
    tm = nc.alloc_semaphore("tm")      # PE transpose done
    dn2 = nc.alloc_semaphore("dn2")    # transposed result in SBUF
    vv = nc.alloc_semaphore("vv")      # vector-engine same-engine RAW ordering

    with nc.Block() as block:

        @block.sync
        def _(sync):
            sync.dma_start(idx_sb[:], idx[:]).then_inc(io_idx, 16)
            sync.dma_start(dnt_sb[:], dnt[:]).then_inc(io_dv, 16)
            sync.dma_start(vdx_sb[:], vdx[:]).then_inc(io_dv, 16)
            sync.dma_start(id_sb[:], ident[:]).then_inc(io_dv, 16)
            sync.wait_ge(dn2, 1)
            sync.dma_start(outT[:], otT_sb[:]).then_inc(st, 16)
            sync.wait_ge(st, 16)

        @block.gpsimd
        def _(gpsimd):
            from concourse import library_config as lc

            gpsimd.load_library(lc.mlp)
            gpsimd.wait_ge(io_idx, 16)
            qn = 0
            for t in range(NT):
                for h in range(2):
                    g3 = g_sb[t][h].rearrange("p (c e) -> p c e", e=E)
                    col = (2 * t + h) * HC
                    for k, (c0, nf) in enumerate(SPLITS):
                        gpsimd.dma_gather(
                            g3[:, c0:c0 + nf, :],
                            table[:],
                            idx_sb[:, col + c0 * 8:col + (c0 + nf) * 8],
                            nf * P,
                            nf * P,
                            E,
                            single_packet=False,
                            queue_num=qn % 4,
                        ).then_inc(gs[t][h][k], 16)
                        qn += 1

        @block.tensor
        def _(tensor):
            tensor.wait_ge(io_dv, 48)
            for t in range(NT):
                nc.tensor.matmul(
                    out=ps_ps[t][:],
                    lhsT=dnt_sb[:, t * P:(t + 1) * P],
                    rhs=vdx_sb[:],
                    start=True,
                    stop=True,
                ).then_inc(mm, 1)
            tensor.wait_ge(dn, NT)
            nc.tensor.matmul(
                out=pst_ps[:], lhsT=ot4_sb[:], rhs=id_sb[:],
                start=True, stop=True,
            ).then_inc(tm, 1)

        @block.vector
        def _(vector):
            W = DW  # 313-wide views skip the pad columns
            # vv counts completed vector ops (13 per tile) so dependent
            # same-engine ops can wait out the deep DVE pipeline
            for t in range(NT):
                base = 11 * t
                s = t % 2
                a3h = []
                g3h = []
                for h in range(2):
                    for k in range(len(SPLITS)):
                        vector.wait_ge(gs[t][h][k], 16)
                    g3 = g_sb[t][h].rearrange("p (c e) -> p c e", e=E)
                    a3 = a_sb[s][h].rearrange("p (c e) -> p c e", e=E)
                    # op 1/2: L1 adds for the two halves (independent)
                    nc.vector.tensor_tensor(
                        out=a3[:, 0:6, 0:W], in0=g3[:, 0:6, 0:W],
                        in1=g3[:, 6:12, 0:W], op=add_op,
                    ).then_inc(vv, 1)
                    a3h.append(a3)
                    g3h.append(g3)
                for h in range(2):
                    # op 3/4: L2 in-place, needs op 1/2 respectively
                    vector.wait_ge(vv, base + 1 + h)
                    nc.vector.tensor_tensor(
                        out=a3h[h][:, 0:3, 0:W], in0=a3h[h][:, 0:3, 0:W],
                        in1=a3h[h][:, 3:6, 0:W], op=add_op,
                    ).then_inc(vv, 1)
                a03, a13 = a3h
                g03, g13 = g3h
                chain = [
                    (a03[:, 0:3, 0:W], a13[:, 0:3, 0:W]),  # op 5
                    (a03[:, 0, 0:W], a03[:, 1, 0:W]),      # op 6
                    (a03[:, 0, 0:W], a03[:, 2, 0:W]),      # op 7
                    (a03[:, 0, 0:W], g03[:, 12, 0:W]),     # op 8
                    (a03[:, 0, 0:W], g13[:, 12, 0:W]),     # op 9
                ]
                for j, (dst, src_) in enumerate(chain):
                    vector.wait_ge(vv, base + 4 + j)
                    nc.vector.tensor_tensor(
                        out=dst, in0=dst, in1=src_, op=add_op,
                    ).then_inc(vv, 1)
                vector.wait_ge(mm, t + 1)
                vector.wait_ge(vv, base + 9)
                nc.vector.tensor_tensor(       # op 10
                    out=tot_sb[t][:], in0=a_sb[s][0][:, :DW],
                    in1=ps_ps[t][:], op=add_op,
                ).then_inc(vv, 1)
                tv = tot_sb[t][:, :D].rearrange("p (i k) -> p k i", k=K)
                vector.wait_ge(vv, base + 10)
                nc.vector.reduce_sum(          # op 11 (inc carried by ve)
                    out=s8_sb[t][:], in_=tv, axis=mybir.AxisListType.X
                ).then_inc(ve, 1)
                vector.wait_ge(ac, 2 * (t + 1))
                nc.vector.tensor_tensor(       # op 12
                    out=ot_sb[t], in0=h2_sb[t][:], in1=h1_sb[t][:],
                    op=mybir.AluOpType.subtract,
                ).then_inc(vv, 1)
                vector.wait_ge(vv, base + 11)
                nc.vector.tensor_tensor(       # op 13 (inc carried by dn)
                    out=ot_sb[t], in0=ot_sb[t], in1=tot_sb[t][:, D:DW],
                    op=add_op,
                ).then_inc(dn, 1)
            vector.wait_ge(tm, 1)
            nc.vector.tensor_copy(out=otT_sb[:], in_=pst_ps[:]).then_inc(dn2, 1)

        @block.scalar
        def _(scalar):
            for t in range(NT):
                s = t % 2
                scalar.wait_ge(ve, t + 1)
                nc.scalar.activation(
                    out=sq_sb[s][:], in_=tot_sb[t][:, :D],
                    func=mybir.ActivationFunctionType.Square,
                    scale=SQRT_HALF, accum_out=h1_sb[t][:],
                ).then_inc(ac, 1)
                nc.scalar.activation(
                    out=sq8_sb[s][:], in_=s8_sb[t][:],
                    func=mybir.ActivationFunctionType.Square,
                    scale=SQRT_HALF, accum_out=h2_sb[t][:],
                ).then_inc(ac, 1)

    nc.compile()
    _cached_nc = nc
    return nc


def _prepare_inputs(inputs, w0, w, v):
    dense = np.ascontiguousarray(inputs[:, :N_DENSE].astype(np.float32))
    idx = inputs[:, N_DENSE:].astype(np.int32)
    flat_idx = (N_DENSE + np.arange(N_SPARSE, dtype=np.int32) * ONEHOT)[None, :] + idx

    table = np.zeros((FEAT, E), np.float16)
    table[:, :D] = v.reshape(FEAT, D).astype(np.float16)
    table[:, D] = np.asarray(w, np.float32).reshape(FEAT).astype(np.float16)
    w0_row = np.zeros((1, DW), np.float32)
    w0_row[0, D] = np.asarray(w0, np.float32).reshape(-1)[0]
    vdx_top = np.concatenate(
        [v.reshape(FEAT, D)[:N_DENSE], np.asarray(w, np.float32).reshape(FEAT, 1)[:N_DENSE]],
        axis=1,
    ).astype(np.float32)
    vdx = np.ascontiguousarray(np.concatenate([vdx_top, w0_row], axis=0))
    ident = np.eye(P, dtype=np.float32)

    in_maps = []
    for c in range(NCORES):
        sl = slice(c * BC, (c + 1) * BC)
        dnt = np.concatenate(
            [dense[sl].T, np.ones((1, BC), np.float32)], axis=0
        )  # [14, 512]
        # per tile t and half h the gathers consume indices i = c_local*128+p,
        # laid out int16 at [i % 16, i // 16] in the first 16 partitions,
        # replicated 8x down the partitions (one copy per Q7 core)
        fi = flat_idx[sl].astype(np.int16)  # [512, 26]
        blocks = []
        for t in range(NT):
            for h in range(2):
                lin = fi[t * P:(t + 1) * P, 13 * h:13 * (h + 1)].T.reshape(NI // 2)
                blk = lin.reshape(NI // 32, 16).T  # [16, HC]
                blocks.append(np.tile(blk, (8, 1)))  # [128, HC]
        idx_buf = np.ascontiguousarray(np.concatenate(blocks, axis=1))
        in_maps.append(
            {
                "table": table,
                "idx": idx_buf,
                "dnt": np.ascontiguousarray(dnt),
                "vdx": vdx,
                "ident": ident,
            }
        )
    return in_maps


def kernel(**inputs):
    from concourse import bass_utils

    nc = _build_program()
    in_maps = _prepare_inputs(
        np.asarray(inputs["inputs"]),
        np.asarray(inputs["w0"]),
        np.asarray(inputs["w"]),
        np.asarray(inputs["v"]),
    )
    res = bass_utils.run_bass_kernel_spmd(nc, in_maps, core_ids=list(range(NCORES)))
    outs = [np.asarray(res.results[c]["out"]) for c in range(NCORES)]
    return np.concatenate(outs, axis=0).astype(np.float32)


# revision 32
# speedup vs baseline: 1.2229x; 1.1398x over previous
"""FFM layer kernel for Trainium2, data-parallel over batch on 8 NeuronCores.

The reference computes, for each sample b:
    x = [dense(13) | onehot(26 fields x 1000)]            # [B, 26013]
    linear = w0 + x @ w                                   # [B, 1]
    field_f = einsum('bf,fik->bik', x, v)                 # [B, 39, 8]
    inter = 0.5*((sum_i field_f)^2.sum(k) - (field_f^2).sum(i,k))
    out = linear + inter

Because x is one-hot in the sparse block, x @ [v|w] is a 26-row gather from
an fp16 [26013, 384] table (cols 0..311 = flattened v row, col 312 = w,
313.. pad so each row is 768 B, a multiple of 256) plus a tiny fp32 dense
[14]x[14,313] matmul (row 13 = ones row carrying w0 into col 312).  Each
core handles 512 samples as 4 tiles of 128; each tile's 26 rows/sample are
fetched by six dma_gather calls (5/4/4 fields per 13-field half) spread
over 4 SWDGE queues.  Raw bacc with manual semaphores — no TileContext —
to avoid the ~35 us Tile prelude/epilogue.
"""

import numpy as np

N_DENSE = 13
N_SPARSE = 26
ONEHOT = 1000
FIELD = 39
K = 8
FEAT = N_DENSE + N_SPARSE * ONEHOT  # 26013
B = 4096
NCORES = 8
BC = B // NCORES  # 512 samples per core
P = 128
NT = BC // P  # 4 tiles per core
D = FIELD * K  # 312
DW = D + 1  # 313 (col 312 carries the linear weight)
E = 384  # gathered fp16 row width (768 B, multiple of 256)
NI = N_SPARSE * P  # 3328 gathered rows per tile
HC = (NI // 2) // 16  # idx columns per 13-field half
SPLITS = ((0, 5), (5, 4), (9, 4))  # sub-gathers per half
SQRT_HALF = 0.7071067811865476

_cached_nc = None


def _build_program():
    global _cached_nc
    if _cached_nc is not None:
        return _cached_nc

    import concourse.bacc as bacc
    import concourse.mybir as mybir
    from concourse import library_config

    nc = bacc.Bacc(
        "TRN2",
        debug=False,
        enable_asserts=False,
        target_bir_lowering=False,
        num_devices=NCORES,
        num_swdge_queues=4,
        dynamic_dma_scratch_size=32768,
    )
    f32 = mybir.dt.float32
    f16 = mybir.dt.float16
    i16 = mybir.dt.int16
    add_op = mybir.AluOpType.add

    table = nc.dram_tensor("table", [FEAT, E], f16, kind="ExternalInput").ap()
    idx = nc.dram_tensor("idx", [P, NT * NI // 16], i16, kind="ExternalInput").ap()
    dnt = nc.dram_tensor("dnt", [N_DENSE + 1, BC], f32, kind="ExternalInput").ap()
    vdx = nc.dram_tensor("vdx", [N_DENSE + 1, DW], f32, kind="ExternalInput").ap()
    ident = nc.dram_tensor("ident", [P, P], f32, kind="ExternalInput").ap()
    out = nc.dram_tensor("out", [BC, 1], f32, kind="ExternalOutput").ap()
    outT = out.rearrange("(t p) o -> t (p o)", t=NT)

    idx_sb = nc.alloc_sbuf_tensor("idx_sb", [P, NT * NI // 16], i16).ap()
    dnt_sb = nc.alloc_sbuf_tensor("dnt_sb", [N_DENSE + 1, BC], f32).ap()
    vdx_sb = nc.alloc_sbuf_tensor("vdx_sb", [N_DENSE + 1, DW], f32).ap()
    id_sb = nc.alloc_sbuf_tensor("id_sb", [P, P], f32).ap()
    g_sb = [
        [nc.alloc_sbuf_tensor(f"g{t}_{h}", [P, 13 * E], f16).ap() for h in range(2)]
        for t in range(NT)
    ]
    # two alternating fp32 partial-sum sets (vector-engine-private, so the
    # engine's own program order serializes reuse)
    a_sb = [
        [nc.alloc_sbuf_tensor(f"a{s}_{h}", [P, 6 * E], f16).ap() for h in range(2)]
        for s in range(2)
    ]
    tot_sb = [nc.alloc_sbuf_tensor(f"tot{t}", [P, DW], f32).ap() for t in range(NT)]
    s8_sb = [nc.alloc_sbuf_tensor(f"s8_{t}", [P, K], f32).ap() for t in range(NT)]
    sq_sb = [nc.alloc_sbuf_tensor(f"sq{s}", [P, D], f32).ap() for s in range(2)]
    sq8_sb = [nc.alloc_sbuf_tensor(f"sq8_{s}", [P, K], f32).ap() for s in range(2)]
    h1_sb = [nc.alloc_sbuf_tensor(f"h1_{t}", [P, 1], f32).ap() for t in range(NT)]
    h2_sb = [nc.alloc_sbuf_tensor(f"h2_{t}", [P, 1], f32).ap() for t in range(NT)]
    ot4_sb = nc.alloc_sbuf_tensor("ot4", [P, NT], f32).ap()
    ot_sb = [ot4_sb[:, t:t + 1] for t in range(NT)]
    otT_sb = nc.alloc_sbuf_tensor("otT", [NT, P], f32).ap()
    ps_ps = [nc.alloc_psum_tensor(f"ps{t}", [P, DW], f32).ap() for t in range(NT)]
    pst_ps = nc.alloc_psum_tensor("psT", [NT, P], f32).ap()

    io_idx = nc.alloc_semaphore("io_idx")  # idx load x 16
    io_dv = nc.alloc_semaphore("io_dv")    # dnt+vdx loads x 16 each
    st = nc.alloc_semaphore("st")      # output stores x 16
    # one sem per sub-gather: a DMA sem may only be updated from one SWDGE
    # queue, so the 3 sub-gathers of a tile-half can't share one
    gs = [
        [[nc.alloc_semaphore(f"gs{t}_{h}_{k}") for k in range(len(SPLITS))]
         for h in range(2)]
        for t in range(NT)
    ]
    mm = nc.alloc_semaphore("mm")      # matmul done (per tile)
    ve = nc.alloc_semaphore("ve")      # tot+s8 ready for scalar engine
    ac = nc.alloc_semaphore("ac")      # scalar squares done (2 per tile)
    dn = nc.alloc_semaphore("dn")      # ot ready for store
    tm = nc.alloc_semaphore("tm")      # PE transpose done
    dn2 = nc.alloc_semaphore("dn2")    # transposed result in SBUF
    vv = nc.alloc_semaphore("vv")      # vector-engine same-engine RAW ordering

    with nc.Block() as block:

        @block.sync
        def _(sync):
            sync.dma_start(idx_sb[:], idx[:]).then_inc(io_idx, 16)
            sync.dma_start(dnt_sb[:], dnt[:]).then_inc(io_dv, 16)
            sync.dma_start(vdx_sb[:], vdx[:]).then_inc(io_dv, 16)
            sync.dma_start(id_sb[:], ident[:]).then_inc(io_dv, 16)
            sync.wait_ge(dn2, 1)
            sync.dma_start(outT[:], otT_sb[:]).then_inc(st, 16)
            sync.wait_ge(st, 16)

        @block.gpsimd
        def _(gpsimd):
            from concourse import library_config as lc

            gpsimd.load_library(lc.mlp)
            gpsimd.wait_ge(io_idx, 16)
            qn = 0
            for t in range(NT):
                for h in range(2):
                    g3 = g_sb[t][h].rearrange("p (c e) -> p c e", e=E)
                    col = (2 * t + h) * HC
                    for k, (c0, nf) in enumerate(SPLITS):
                        gpsimd.dma_gather(
                            g3[:, c0:c0 + nf, :],
                            table[:],
                            idx_sb[:, col + c0 * 8:col + (c0 + nf) * 8],
                            nf * P,
                            nf * P,
                            E,
                            single_packet=False,
                            queue_num=qn % 4,
                        ).then_inc(gs[t][h][k], 16)
                        qn += 1

        @block.tensor
        def _(tensor):
            tensor.wait_ge(io_dv, 48)
            for t in range(NT):
                nc.tensor.matmul(
                    out=ps_ps[t][:],
                    lhsT=dnt_sb[:, t * P:(t + 1) * P],
                    rhs=vdx_sb[:],
                    start=True,
                    stop=True,
                ).then_inc(mm, 1)
            tensor.wait_ge(dn, NT)
            nc.tensor.matmul(
                out=pst_ps[:], lhsT=ot4_sb[:], rhs=id_sb[:],
                start=True, stop=True,
            ).then_inc(tm, 1)

        @block.vector
        def _(vector):
            W = DW       # 313 payload columns
            W2 = DW + 1  # even width so fp16 adds can take the 2x read-packed mode
            # vv counts completed vector ops (9 per tile) so dependent
            # same-engine ops can wait out the deep DVE pipeline
            for t in range(NT):
                base = 9 * t
                s = t % 2
                a3h = []
                g3h = []
                for h in range(2):
                    for k in range(len(SPLITS)):
                        vector.wait_ge(gs[t][h][k], 16)
                    g3 = g_sb[t][h].rearrange("p (c e) -> p c e", e=E)
                    a3 = a_sb[s][h].rearrange("p (c e) -> p c e", e=E)
                    # op 1/2: fp16 L1 adds for the two halves
                    nc.vector.tensor_tensor(
                        out=a3[:, 0:6, 0:W2], in0=g3[:, 0:6, 0:W2],
                        in1=g3[:, 6:12, 0:W2], op=add_op,
                    ).then_inc(vv, 1)
                    a3h.append(a3)
                    g3h.append(g3)
                for h in range(2):
                    # op 3/4: fp16 L2 in-place
                    vector.wait_ge(vv, base + 1 + h)
                    nc.vector.tensor_tensor(
                        out=a3h[h][:, 0:3, 0:W2], in0=a3h[h][:, 0:3, 0:W2],
                        in1=a3h[h][:, 3:6, 0:W2], op=add_op,
                    ).then_inc(vv, 1)
                a03, a13 = a3h
                g03, g13 = g3h
                # op 5: 13th blocks folded into a03 block 3 (freed by op 3)
                vector.wait_ge(vv, base + 3)
                nc.vector.tensor_tensor(
                    out=a03[:, 3, 0:W2], in0=g03[:, 12, 0:W2],
                    in1=g13[:, 12, 0:W2], op=add_op,
                ).then_inc(vv, 1)
                # op 6: cross-half add into a03 blocks 0..2
                vector.wait_ge(vv, base + 4)
                nc.vector.tensor_tensor(
                    out=a03[:, 0:3, 0:W2], in0=a03[:, 0:3, 0:W2],
                    in1=a13[:, 0:3, 0:W2], op=add_op,
                ).then_inc(vv, 1)
                # op 7: reduce the 4 fp16 blocks to the fp32 total
                r4 = a_sb[s][0][:, :4 * E].rearrange("p (c e) -> p e c", e=E)
                vector.wait_ge(vv, base + 6)
                nc.vector.reduce_sum(
                    out=tot_sb[t][:], in_=r4[:, 0:W, :], axis=mybir.AxisListType.X
                ).then_inc(vv, 1)
                # op 8: add the dense+w0 matmul part
                vector.wait_ge(mm, t + 1)
                vector.wait_ge(vv, base + 7)
                nc.vector.tensor_tensor(
                    out=tot_sb[t][:], in0=tot_sb[t][:], in1=ps_ps[t][:],
                    op=add_op,
                ).then_inc(vv, 1)
                # op 9: s_k = sum_i f_ik (completion carried by ve)
                tv = tot_sb[t][:, :D].rearrange("p (i k) -> p k i", k=K)
                vector.wait_ge(vv, base + 8)
                nc.vector.reduce_sum(
                    out=s8_sb[t][:], in_=tv, axis=mybir.AxisListType.X
                ).then_inc(ve, 1)
                # op 10/11: combine with the scalar-engine squares
                vector.wait_ge(ac, 2 * (t + 1))
                nc.vector.tensor_tensor(
                    out=ot_sb[t], in0=h2_sb[t][:], in1=h1_sb[t][:],
                    op=mybir.AluOpType.subtract,
                ).then_inc(vv, 1)
                vector.wait_ge(vv, base + 9)
                nc.vector.tensor_tensor(
                    out=ot_sb[t], in0=ot_sb[t], in1=tot_sb[t][:, D:DW],
                    op=add_op,
                ).then_inc(dn, 1)
            vector.wait_ge(tm, 1)
            nc.vector.tensor_copy(out=otT_sb[:], in_=pst_ps[:]).then_inc(dn2, 1)

        @block.scalar
        def _(scalar):
            for t in range(NT):
                s = t % 2
                scalar.wait_ge(ve, t + 1)
                nc.scalar.activation(
                    out=sq_sb[s][:], in_=tot_sb[t][:, :D],
                    func=mybir.ActivationFunctionType.Square,
                    scale=SQRT_HALF, accum_out=h1_sb[t][:],
                ).then_inc(ac, 1)
                nc.scalar.activation(
                    out=sq8_sb[s][:], in_=s8_sb[t][:],
                    func=mybir.ActivationFunctionType.Square,
                    scale=SQRT_HALF, accum_out=h2_sb[t][:],
                ).then_inc(ac, 1)

    nc.compile()
    _cached_nc = nc
    return nc


def _prepare_inputs(inputs, w0, w, v):
    dense = np.ascontiguousarray(inputs[:, :N_DENSE].astype(np.float32))
    idx = inputs[:, N_DENSE:].astype(np.int32)
    flat_idx = (N_DENSE + np.arange(N_SPARSE, dtype=np.int32) * ONEHOT)[None, :] + idx

    table = np.zeros((FEAT, E), np.float16)
    table[:, :D] = v.reshape(FEAT, D).astype(np.float16)
    table[:, D] = np.asarray(w, np.float32).reshape(FEAT).astype(np.float16)
    w0_row = np.zeros((1, DW), np.float32)
    w0_row[0, D] = np.asarray(w0, np.float32).reshape(-1)[0]
    vdx_top = np.concatenate(
        [v.reshape(FEAT, D)[:N_DENSE], np.asarray(w, np.float32).reshape(FEAT, 1)[:N_DENSE]],
        axis=1,
    ).astype(np.float32)
    vdx = np.ascontiguousarray(np.concatenate([vdx_top, w0_row], axis=0))
    ident = np.eye(P, dtype=np.float32)

    in_maps = []
    for c in range(NCORES):
        sl = slice(c * BC, (c + 1) * BC)
        dnt = np.concatenate(
            [dense[sl].T, np.ones((1, BC), np.float32)], axis=0
        )  # [14, 512]
        # per tile t and half h the gathers consume indices i = c_local*128+p,
        # laid out int16 at [i % 16, i // 16] in the first 16 partitions,
        # replicated 8x down the partitions (one copy per Q7 core)
        fi = flat_idx[sl].astype(np.int16)  # [512, 26]
        blocks = []
        for t in range(NT):
            for h in range(2):
                lin = fi[t * P:(t + 1) * P, 13 * h:13 * (h + 1)].T.reshape(NI // 2)
                blk = lin.reshape(NI // 32, 16).T  # [16, HC]
                blocks.append(np.tile(blk, (8, 1)))  # [128, HC]
        idx_buf = np.ascontiguousarray(np.concatenate(blocks, axis=1))
        in_maps.append(
            {
                "table": table,
                "idx": idx_buf,
                "dnt": np.ascontiguousarray(dnt),
                "vdx": vdx,
                "ident": ident,
            }
        )
    return in_maps


def kernel(**inputs):
    from concourse import bass_utils

    nc = _build_program()
    in_maps = _prepare_inputs(
        np.asarray(inputs["inputs"]),
        np.asarray(inputs["w0"]),
        np.asarray(inputs["w"]),
        np.asarray(inputs["v"]),
    )
    res = bass_utils.run_bass_kernel_spmd(nc, in_maps, core_ids=list(range(NCORES)))
    outs = [np.asarray(res.results[c]["out"]) for c in range(NCORES)]
    return np.concatenate(outs, axis=0).astype(np.float32)


# revision 39
# speedup vs baseline: 1.2823x; 1.0486x over previous
"""FFM layer kernel for Trainium2, data-parallel over batch on 8 NeuronCores.

The reference computes, for each sample b:
    x = [dense(13) | onehot(26 fields x 1000)]            # [B, 26013]
    linear = w0 + x @ w                                   # [B, 1]
    field_f = einsum('bf,fik->bik', x, v)                 # [B, 39, 8]
    inter = 0.5*((sum_i field_f)^2.sum(k) - (field_f^2).sum(i,k))
    out = linear + inter

Because x is one-hot in the sparse block, x @ [v|w] is a 26-row gather from
an fp16 [26013, 384] table (cols 0..311 = flattened v row, col 312 = w,
313.. pad so each row is 768 B, a multiple of 256) plus a tiny fp32 dense
[14]x[14,313] matmul (row 13 = ones row carrying w0 into col 312).  Each
core handles 512 samples as 4 tiles of 128; each tile's 26 rows/sample are
fetched by six dma_gather calls (5/4/4 fields per 13-field half) spread
over 4 SWDGE queues.  Raw bacc with manual semaphores — no TileContext.
The 26 gathered rows are summed with an fp16 pairwise tree on the vector
engine (even-width views so the 2x read-packed mode engages); the
interaction terms stay on the vector engine too (square / reduce /
fused tensor_scalar), avoiding cross-engine semaphore hops on the tail;
the [512,1] result is PE-transposed against a host-supplied identity so
the final store is 4 contiguous 512 B rows instead of 512
partition-strided words.
"""

import numpy as np

N_DENSE = 13
N_SPARSE = 26
ONEHOT = 1000
FIELD = 39
K = 8
FEAT = N_DENSE + N_SPARSE * ONEHOT  # 26013
B = 4096
NCORES = 8
BC = B // NCORES  # 512 samples per core
P = 128
NT = BC // P  # 4 tiles per core
D = FIELD * K  # 312
DW = D + 1  # 313 (col 312 carries the linear weight)
E = 384  # gathered fp16 row width (768 B, multiple of 256)
NI = N_SPARSE * P  # 3328 gathered rows per tile
HC = (NI // 2) // 16  # idx columns per 13-field half
SPLITS = ((0, 5), (5, 4), (9, 4))  # sub-gathers per half
SQRT_HALF = 0.7071067811865476

_cached_nc = None


def _build_program():
    global _cached_nc
    if _cached_nc is not None:
        return _cached_nc

    import concourse.bacc as bacc
    import concourse.mybir as mybir
    from concourse import library_config

    nc = bacc.Bacc(
        "TRN2",
        debug=False,
        enable_asserts=False,
        target_bir_lowering=False,
        num_devices=NCORES,
        num_swdge_queues=4,
        dynamic_dma_scratch_size=65536,
    )
    f32 = mybir.dt.float32
    f16 = mybir.dt.float16
    i16 = mybir.dt.int16
    add_op = mybir.AluOpType.add

    table = nc.dram_tensor("table", [FEAT, E], f16, kind="ExternalInput").ap()
    idx = nc.dram_tensor("idx", [P, NT * NI // 16], i16, kind="ExternalInput").ap()
    dnt = nc.dram_tensor("dnt", [N_DENSE + 1, BC], f32, kind="ExternalInput").ap()
    vdx = nc.dram_tensor("vdx", [N_DENSE + 1, DW], f32, kind="ExternalInput").ap()
    ident = nc.dram_tensor("ident", [P, P], f32, kind="ExternalInput").ap()
    out = nc.dram_tensor("out", [BC, 1], f32, kind="ExternalOutput").ap()
    outT = out.rearrange("(t p) o -> t (p o)", t=NT)

    idx_sb = nc.alloc_sbuf_tensor("idx_sb", [P, NT * NI // 16], i16).ap()
    dnt_sb = nc.alloc_sbuf_tensor("dnt_sb", [N_DENSE + 1, BC], f32).ap()
    vdx_sb = nc.alloc_sbuf_tensor("vdx_sb", [N_DENSE + 1, DW], f32).ap()
    id_sb = nc.alloc_sbuf_tensor("id_sb", [P, P], f32).ap()
    g_sb = [
        [nc.alloc_sbuf_tensor(f"g{t}_{h}", [P, 13 * E], f16).ap() for h in range(2)]
        for t in range(NT)
    ]
    # two alternating fp32 partial-sum sets (vector-engine-private, so the
    # engine's own program order serializes reuse)
    a_sb = [
        [nc.alloc_sbuf_tensor(f"a{s}_{h}", [P, 6 * E], f16).ap() for h in range(2)]
        for s in range(2)
    ]
    tot_sb = [nc.alloc_sbuf_tensor(f"tot{t}", [P, DW], f32).ap() for t in range(NT)]
    s8_sb = [nc.alloc_sbuf_tensor(f"s8_{t}", [P, K], f32).ap() for t in range(NT)]
    sq_sb = [nc.alloc_sbuf_tensor(f"sq{s}", [P, D], f32).ap() for s in range(2)]
    sq8_sb = [nc.alloc_sbuf_tensor(f"sq8_{s}", [P, K], f32).ap() for s in range(2)]
    h1_sb = [nc.alloc_sbuf_tensor(f"h1_{t}", [P, 1], f32).ap() for t in range(NT)]
    h2_sb = [nc.alloc_sbuf_tensor(f"h2_{t}", [P, 1], f32).ap() for t in range(NT)]
    rr_sb = [nc.alloc_sbuf_tensor(f"rr_{t}", [P, 1], f32).ap() for t in range(NT)]
    ot4_sb = nc.alloc_sbuf_tensor("ot4", [P, NT], f32).ap()
    ot_sb = [ot4_sb[:, t:t + 1] for t in range(NT)]
    otT_sb = nc.alloc_sbuf_tensor("otT", [NT, P], f32).ap()
    ps_ps = [nc.alloc_psum_tensor(f"ps{t}", [P, DW], f32).ap() for t in range(NT)]
    pst_ps = nc.alloc_psum_tensor("psT", [NT, P], f32).ap()

    io_idx = nc.alloc_semaphore("io_idx")  # idx load x 16
    io_dv = nc.alloc_semaphore("io_dv")    # dnt+vdx loads x 16 each
    st = nc.alloc_semaphore("st")      # output stores x 16
    # one sem per sub-gather: a DMA sem may only be updated from one SWDGE
    # queue, so the 3 sub-gathers of a tile-half can't share one
    gs = [
        [[nc.alloc_semaphore(f"gs{t}_{h}_{k}") for k in range(len(SPLITS))]
         for h in range(2)]
        for t in range(NT)
    ]
    mm = nc.alloc_semaphore("mm")      # matmul done (per tile)
    dn = nc.alloc_semaphore("dn")      # ot ready for store
    tm = nc.alloc_semaphore("tm")      # PE transpose done
    dn2 = nc.alloc_semaphore("dn2")    # transposed result in SBUF
    vv = nc.alloc_semaphore("vv")      # vector-engine same-engine RAW ordering

    with nc.Block() as block:

        @block.sync
        def _(sync):
            sync.dma_start(idx_sb[:], idx[:]).then_inc(io_idx, 16)
            sync.dma_start(dnt_sb[:], dnt[:]).then_inc(io_dv, 16)
            sync.dma_start(vdx_sb[:], vdx[:]).then_inc(io_dv, 16)
            sync.dma_start(id_sb[:], ident[:]).then_inc(io_dv, 16)
            sync.wait_ge(dn2, 1)
            sync.dma_start(outT[:], otT_sb[:]).then_inc(st, 16)
            sync.wait_ge(st, 16)

        @block.gpsimd
        def _(gpsimd):
            from concourse import library_config as lc

            gpsimd.load_library(lc.mlp)
            gpsimd.wait_ge(io_idx, 16)
            qn = 0
            for t in range(NT):
                for h in range(2):
                    g3 = g_sb[t][h].rearrange("p (c e) -> p c e", e=E)
                    col = (2 * t + h) * HC
                    for k, (c0, nf) in enumerate(SPLITS):
                        gpsimd.dma_gather(
                            g3[:, c0:c0 + nf, :],
                            table[:],
                            idx_sb[:, col + c0 * 8:col + (c0 + nf) * 8],
                            nf * P,
                            nf * P,
                            E,
                            single_packet=False,
                            queue_num=qn % 4,
                        ).then_inc(gs[t][h][k], 16)
                        qn += 1

        @block.tensor
        def _(tensor):
            tensor.wait_ge(io_dv, 48)
            for t in range(NT):
                nc.tensor.matmul(
                    out=ps_ps[t][:],
                    lhsT=dnt_sb[:, t * P:(t + 1) * P],
                    rhs=vdx_sb[:],
                    start=True,
                    stop=True,
                ).then_inc(mm, 1)
            tensor.wait_ge(dn, NT)
            nc.tensor.matmul(
                out=pst_ps[:], lhsT=ot4_sb[:], rhs=id_sb[:],
                start=True, stop=True,
            ).then_inc(tm, 1)

        @block.vector
        def _(vector):
            W = DW       # 313 payload columns
            W2 = DW + 1  # even width so fp16 adds can take the 2x read-packed mode
            # vv counts completed vector ops (9 per tile) so dependent
            # same-engine ops can wait out the deep DVE pipeline
            for t in range(NT):
                base = 16 * t
                s = t % 2
                a3h = []
                g3h = []
                for h in range(2):
                    for k in range(len(SPLITS)):
                        vector.wait_ge(gs[t][h][k], 16)
                    g3 = g_sb[t][h].rearrange("p (c e) -> p c e", e=E)
                    a3 = a_sb[s][h].rearrange("p (c e) -> p c e", e=E)
                    # op 1/2: fp16 L1 adds for the two halves
                    nc.vector.tensor_tensor(
                        out=a3[:, 0:6, 0:W2], in0=g3[:, 0:6, 0:W2],
                        in1=g3[:, 6:12, 0:W2], op=add_op,
                    ).then_inc(vv, 1)
                    a3h.append(a3)
                    g3h.append(g3)
                for h in range(2):
                    # op 3/4: fp16 L2 in-place
                    vector.wait_ge(vv, base + 1 + h)
                    nc.vector.tensor_tensor(
                        out=a3h[h][:, 0:3, 0:W2], in0=a3h[h][:, 0:3, 0:W2],
                        in1=a3h[h][:, 3:6, 0:W2], op=add_op,
                    ).then_inc(vv, 1)
                a03, a13 = a3h
                g03, g13 = g3h
                # op 5: 13th blocks folded into a03 block 3 (freed by op 3)
                vector.wait_ge(vv, base + 3)
                nc.vector.tensor_tensor(
                    out=a03[:, 3, 0:W2], in0=g03[:, 12, 0:W2],
                    in1=g13[:, 12, 0:W2], op=add_op,
                ).then_inc(vv, 1)
                # op 6: cross-half add into a03 blocks 0..2
                vector.wait_ge(vv, base + 4)
                nc.vector.tensor_tensor(
                    out=a03[:, 0:3, 0:W2], in0=a03[:, 0:3, 0:W2],
                    in1=a13[:, 0:3, 0:W2], op=add_op,
                ).then_inc(vv, 1)
                # ops 7-8: fp16 pairwise collapse of the 4 blocks
                vector.wait_ge(vv, base + 6)
                nc.vector.tensor_tensor(
                    out=a03[:, 0, 0:W2], in0=a03[:, 0, 0:W2],
                    in1=a03[:, 1, 0:W2], op=add_op,
                ).then_inc(vv, 1)
                vector.wait_ge(vv, base + 6)
                nc.vector.tensor_tensor(
                    out=a03[:, 2, 0:W2], in0=a03[:, 2, 0:W2],
                    in1=a03[:, 3, 0:W2], op=add_op,
                ).then_inc(vv, 1)
                # op 9: final fp32 total = b0 + b2 + dense part (two adds)
                vector.wait_ge(vv, base + 8)
                nc.vector.tensor_tensor(
                    out=tot_sb[t][:], in0=a03[:, 0, 0:W], in1=a03[:, 2, 0:W],
                    op=add_op,
                ).then_inc(vv, 1)
                # op 10: add the dense+w0 matmul part
                vector.wait_ge(mm, t + 1)
                vector.wait_ge(vv, base + 9)
                nc.vector.tensor_tensor(
                    out=tot_sb[t][:], in0=tot_sb[t][:], in1=ps_ps[t][:],
                    op=add_op,
                ).then_inc(vv, 1)
                # ops 11-17: interaction terms entirely on the vector
                # engine (no scalar-engine round trip on the critical tail);
                # independent ops interleaved so dependent pairs are never
                # back-to-back in the deep DVE pipe
                tv = tot_sb[t][:, :D].rearrange("p (i k) -> p k i", k=K)
                vector.wait_ge(vv, base + 10)
                nc.vector.reduce_sum(          # op 11: s_k = sum_i f_ik
                    out=s8_sb[t][:], in_=tv, axis=mybir.AxisListType.X
                ).then_inc(vv, 1)
                nc.vector.tensor_tensor(       # op 12: sq = f*f
                    out=sq_sb[s][:], in0=tot_sb[t][:, :D], in1=tot_sb[t][:, :D],
                    op=mybir.AluOpType.mult,
                ).then_inc(vv, 1)
                vector.wait_ge(vv, base + 11)
                nc.vector.tensor_tensor(       # op 13: sq8 = s*s
                    out=sq8_sb[s][:], in0=s8_sb[t][:], in1=s8_sb[t][:],
                    op=mybir.AluOpType.mult,
                ).then_inc(vv, 1)
                vector.wait_ge(vv, base + 12)
                nc.vector.reduce_sum(          # op 14: h1 = sum f^2
                    out=h1_sb[t][:], in_=sq_sb[s][:], axis=mybir.AxisListType.X
                ).then_inc(vv, 1)
                vector.wait_ge(vv, base + 13)
                nc.vector.reduce_sum(          # op 15: h2 = sum s^2
                    out=h2_sb[t][:], in_=sq8_sb[s][:], axis=mybir.AxisListType.X
                ).then_inc(vv, 1)
                vector.wait_ge(vv, base + 15)
                nc.vector.tensor_scalar(       # op 16: r = (h2 - h1) * 0.5
                    out=rr_sb[t][:], in0=h2_sb[t][:], scalar1=h1_sb[t][:],
                    scalar2=0.5, op0=mybir.AluOpType.subtract,
                    op1=mybir.AluOpType.mult,
                ).then_inc(vv, 1)
                vector.wait_ge(vv, base + 16)
                nc.vector.tensor_scalar(       # op 17: out = r + linear
                    out=ot_sb[t], in0=rr_sb[t][:], scalar1=tot_sb[t][:, D:DW],
                    scalar2=None, op0=add_op,
                ).then_inc(dn, 1)
            vector.wait_ge(tm, 1)
            nc.vector.tensor_copy(out=otT_sb[:], in_=pst_ps[:]).then_inc(dn2, 1)

        @block.scalar
        def _(scalar):
            scalar.wait_ge(st, 16)

    nc.compile()
    _cached_nc = nc
    return nc


def _prepare_inputs(inputs, w0, w, v):
    dense = np.ascontiguousarray(inputs[:, :N_DENSE].astype(np.float32))
    idx = inputs[:, N_DENSE:].astype(np.int32)
    flat_idx = (N_DENSE + np.arange(N_SPARSE, dtype=np.int32) * ONEHOT)[None, :] + idx

    table = np.zeros((FEAT, E), np.float16)
    table[:, :D] = v.reshape(FEAT, D).astype(np.float16)
    table[:, D] = np.asarray(w, np.float32).reshape(FEAT).astype(np.float16)
    w0_row = np.zeros((1, DW), np.float32)
    w0_row[0, D] = np.asarray(w0, np.float32).reshape(-1)[0]
    vdx_top = np.concatenate(
        [v.reshape(FEAT, D)[:N_DENSE], np.asarray(w, np.float32).reshape(FEAT, 1)[:N_DENSE]],
        axis=1,
    ).astype(np.float32)
    vdx = np.ascontiguousarray(np.concatenate([vdx_top, w0_row], axis=0))
    ident = np.eye(P, dtype=np.float32)

    in_maps = []
    for c in range(NCORES):
        sl = slice(c * BC, (c + 1) * BC)
        dnt = np.concatenate(
            [dense[sl].T, np.ones((1, BC), np.float32)], axis=0
        )  # [14, 512]
        # per tile t and half h the gathers consume indices i = c_local*128+p,
        # laid out int16 at [i % 16, i // 16] in the first 16 partitions,
        # replicated 8x down the partitions (one copy per Q7 core)
        fi = flat_idx[sl].astype(np.int16)  # [512, 26]
        blocks = []
        for t in range(NT):
            for h in range(2):
                lin = fi[t * P:(t + 1) * P, 13 * h:13 * (h + 1)].T.reshape(NI // 2)
                blk = lin.reshape(NI // 32, 16).T  # [16, HC]
                blocks.append(np.tile(blk, (8, 1)))  # [128, HC]
        idx_buf = np.ascontiguousarray(np.concatenate(blocks, axis=1))
        in_maps.append(
            {
                "table": table,
                "idx": idx_buf,
                "dnt": np.ascontiguousarray(dnt),
                "vdx": vdx,
                "ident": ident,
            }
        )
    return in_maps


def kernel(**inputs):
    from concourse import bass_utils

    nc = _build_program()
    in_maps = _prepare_inputs(
        np.asarray(inputs["inputs"]),
        np.asarray(inputs["w0"]),
        np.asarray(inputs["w"]),
        np.asarray(inputs["v"]),
    )
    res = bass_utils.run_bass_kernel_spmd(nc, in_maps, core_ids=list(range(NCORES)))
    outs = [np.asarray(res.results[c]["out"]) for c in range(NCORES)]
    return np.concatenate(outs, axis=0).astype(np.float32)
